# revision 1
# baseline (speedup 1.0000x reference)
"""MoE grouped-GEMM (FMoELinear) on 8 trn2 NeuronCores.

Strategy (expert parallelism):
  - 32 experts, 8 cores -> 4 experts per core.
  - Tokens arrive pre-sorted by expert; host pads each expert's segment to a
    fixed per-expert capacity CAP (multiple of CHUNK) and ships each core a
    transposed activation panel xt[256, 4*CAP] plus its 4 expert weights
    wt[256, 4*256] laid out as [in_feat, expert*256 + out_feat].
  - Device computes yt[o, t] = sum_i W[e][o, i] * x[t, i] per expert with the
    weight stationary in the PE array:
        lhsT = wt[i_chunk, e*256 + oc*128 : +128]   (128 x 128, stationary)
        rhs  = xt tile    [i_chunk, token span]     (128 x 512, moving)
    accumulating the two i-chunks into PSUM, then copies PSUM->SBUF->HBM.
  - Host gathers the non-padded columns back into token order.

The program is identical on all 8 cores (pure SPMD data parallelism); only the
input data differs. All routing logic runs on the host using the runtime
fwd_expert_count values.
"""

import os
import sys
import types

import numpy as np

import concourse.bacc as bacc
import concourse.mybir as mybir
import concourse.tile as tile
from concourse.bass_utils import run_bass_kernel_spmd


def _ensure_axon_hooks_importable():
    """bass_utils imports antenv.axon_hooks when tracing is requested; some
    images lack that module. Provide a no-op fallback so a stray BASS_TRACE
    env var can't crash the kernel (tracing then degrades gracefully)."""
    try:
        import antenv  # noqa: F401
    except ImportError:
        return
    try:
        import antenv.axon_hooks  # noqa: F401
    except ImportError:
        mod = types.ModuleType("antenv.axon_hooks")
        holder = [None]
        mod.set_axon_ntff_profile_hook = lambda h: holder.__setitem__(0, h)
        mod.get_axon_ntff_profile_hook = lambda: holder[0]
        sys.modules["antenv.axon_hooks"] = mod
        import antenv as _antenv

        _antenv.axon_hooks = mod


_ensure_axon_hooks_importable()

NCORES = 8
D = 256  # in/out feature dim
EPC = 4  # experts per core
CHUNK = int(os.environ.get("BASSMOE_CHUNK", "2048"))  # token-span per load
CAPGRAN = 128  # capacity granularity (pad each expert to a multiple of this)

# matmul input dtype: "f32" (exact, 4 cyc/row), "f32r" (1 cyc/row), "f16"/"bf16"
MM_DT = os.environ.get("BASSMOE_MM_DT", "f16")
Y_GPSIMD = bool(int(os.environ.get("BASSMOE_Y_GPSIMD", "0")))
WARM = bool(int(os.environ.get("BASSMOE_WARM", "0")))
FUSE = bool(int(os.environ.get("BASSMOE_FUSE", "1")))
Y_DT = os.environ.get("BASSMOE_Y_DT", "f16")  # output stream dtype
COPY_SPLIT = bool(int(os.environ.get("BASSMOE_COPY_SPLIT", "0")))
XBUFS = int(os.environ.get("BASSMOE_XBUFS", "6"))
YBUFS = int(os.environ.get("BASSMOE_YBUFS", "6"))

# observability for test harness
last_exec_time_ns = None
last_results = None

_prog_cache = {}


def _dt1(name):
    if name == "f32":
        return mybir.dt.float32, np.dtype(np.float32)
    if name == "f32r":
        return mybir.dt.float32r, np.dtype(np.float32)
    if name == "f16":
        return mybir.dt.float16, np.dtype(np.float16)
    if name == "bf16":
        import ml_dtypes

        return mybir.dt.bfloat16, np.dtype(ml_dtypes.bfloat16)
    raise ValueError(name)


def _dtypes():
    """(x_dtype, x_np, w_dtype, w_np); MM_DT may be 'xdt' or 'xdt+wdt'."""
    parts = MM_DT.split("+")
    dx, nx = _dt1(parts[0])
    dw, nw = _dt1(parts[-1])
    return dx, nx, dw, nw


def _chunk_offsets(cap: int):
    """(offset, width) chunks covering [0, cap), width <= CHUNK."""
    out = []
    off = 0
    while off < cap:
        w = min(CHUNK, cap - off)
        out.append((off, w))
        off += w
    return out


def _splits(width: int):
    """(offset, width) matmul spans <= 512 covering [0, width)."""
    out = []
    off = 0
    while off < width:
        w = min(512, width - off)
        out.append((off, w))
        off += w
    return out


def _build_program(cap: int):
    """Build the SPMD Bass program for per-expert capacity `cap` tokens."""
    dt_x, _, dt_w, _ = _dtypes()
    dt_y, _ = _dt1(Y_DT)
    width = EPC * cap

    nc = bacc.Bacc(
        "TRN2",
        target_bir_lowering=False,
        debug=False,
        enable_asserts=False,
        num_devices=NCORES,
    )
    xt = nc.dram_tensor("xt", [D, width], dt_x, kind="ExternalInput").ap()
    wt = nc.dram_tensor("wt", [D, EPC * D], dt_w, kind="ExternalInput").ap()
    yt = nc.dram_tensor("yt", [D, width], dt_y, kind="ExternalOutput").ap()

    with tile.TileContext(nc) as tc:
        with (
            tc.tile_pool(name="w", bufs=1) as wpool,
            tc.tile_pool(name="x", bufs=XBUFS) as xpool,
            tc.tile_pool(name="y", bufs=YBUFS) as ypool,
            tc.tile_pool(name="ps", bufs=7 if WARM else 8, space="PSUM") as pspool,
            tc.tile_pool(name="pw", bufs=1, space="PSUM") as pwpool,
        ):
            # stationary weights for the whole kernel: two i-chunks
            # (loaded via gpsimd so they don't head-of-line block the x loads)
            w0 = wpool.tile([128, EPC * D], dt_w, tag="w0")
            w1 = wpool.tile([128, EPC * D], dt_w, tag="w1")
            nc.gpsimd.dma_start(out=w0[:], in_=wt[0:128, :])
            nc.gpsimd.dma_start(out=w1[:], in_=wt[128:256, :])

            # DRAM views with both 128-row halves on the same 128 partitions
            xt3 = xt.rearrange("(c p) w -> p c w", c=2)
            yt3 = yt.rearrange("(c p) w -> p c w", c=2)

            chidx = 0
            for e in range(EPC):
                for coff, cw in _chunk_offsets(cap):
                    chidx += 1
                    t0 = e * cap + coff
                    if FUSE:
                        x01 = xpool.tile([128, 2 * CHUNK], dt_x, tag="x01")
                        nc.sync.dma_start(
                            out=x01[:].rearrange("p (c w) -> p c w", c=2)[:, :, :cw],
                            in_=xt3[:, :, t0 : t0 + cw],
                        )
                        x0 = x01[:, 0:CHUNK]
                        x1 = x01[:, CHUNK : 2 * CHUNK]
                    else:
                        x0t = xpool.tile([128, CHUNK], dt_x, tag="x0")
                        x1t = xpool.tile([128, CHUNK], dt_x, tag="x1")
                        nc.sync.dma_start(out=x0t[:, :cw], in_=xt[0:128, t0 : t0 + cw])
                        nc.sync.dma_start(
                            out=x1t[:, :cw], in_=xt[128:256, t0 : t0 + cw]
                        )
                        x0 = x0t[:]
                        x1 = x1t[:]
                    if WARM:
                        # tiny matmul tied to this chunk's load keeps the PE's
                        # HAM activity window non-idle (K=8/8, 2.4 GHz)
                        pw = pwpool.tile([128, 8], mybir.dt.float32, tag="warm")
                        nc.tensor.matmul(
                            pw[:], w0[:, 0:128], x0[:, 0:8], start=True, stop=True
                        )
                    ysb01 = None
                    if FUSE:
                        ysb01 = ypool.tile([128, 2 * CHUNK], dt_y, tag="y01")
                    for oc in range(2):
                        col = e * D + oc * 128
                        if FUSE:
                            ysb = ysb01[:, oc * CHUNK : (oc + 1) * CHUNK]
                        else:
                            ysbt = ypool.tile([128, CHUNK], dt_y, tag="y")
                            ysb = ysbt[:]
                        for soff, sw in _splits(cw):
                            ps = pspool.tile([128, 512], mybir.dt.float32, tag="ps")
                            nc.tensor.matmul(
                                ps[:, :sw],
                                w0[:, col : col + 128],
                                x0[:, soff : soff + sw],
                                start=True,
                                stop=False,
                            )
                            nc.tensor.matmul(
                                ps[:, :sw],
                                w1[:, col : col + 128],
                                x1[:, soff : soff + sw],
                                start=False,
                                stop=True,
                            )
                            if COPY_SPLIT and (soff // 512) % 2 == 1:
                                nc.scalar.copy(ysb[:, soff : soff + sw], ps[:, :sw])
                            else:
                                nc.vector.tensor_copy(
                                    ysb[:, soff : soff + sw], ps[:, :sw]
                                )
                        if not FUSE:
                            # stores on the ACT HWDGE ring, loads on the SP ring
                            st_eng = (
                                nc.gpsimd
                                if (Y_GPSIMD and (chidx + oc) % 2)
                                else nc.scalar
                            )
                            st_eng.dma_start(
                                out=yt[oc * 128 : (oc + 1) * 128, t0 : t0 + cw],
                                in_=ysb[:, :cw],
                            )
                    if FUSE:
                        nc.scalar.dma_start(
                            out=yt3[:, :, t0 : t0 + cw],
                            in_=ysb01[:].rearrange("p (c w) -> p c w", c=2)[
                                :, :, :cw
                            ],
                        )
    nc.compile()
    return nc


def kernel(inp, weight, fwd_expert_count, capacity):
    global last_exec_time_ns, last_results

    inp = np.asarray(inp)
    weight = np.asarray(weight)
    counts = np.asarray(fwd_expert_count).astype(np.int64)
    T, d_in = inp.shape
    E = weight.shape[0]
    assert d_in == D and E == NCORES * EPC
    assert int(counts.sum()) == T, "counts must cover all tokens"

    ends = np.cumsum(counts)
    starts = ends - counts
    cap = max(CAPGRAN, int(-(-int(counts.max()) // CAPGRAN)) * CAPGRAN)
    width = EPC * cap

    _, np_x, _, np_w = _dtypes()

    # host-side scatter: transpose once, then contiguous row-slice copies
    xt_full = np.ascontiguousarray(inp.T)  # [D, T] float32
    if np_x != np.float32:
        xt_full = xt_full.astype(np_x)

    in_maps = []
    for dcore in range(NCORES):
        xt = np.zeros((D, width), dtype=np_x)
        for j in range(EPC):
            e = dcore * EPC + j
            s, c = int(starts[e]), int(counts[e])
            xt[:, j * cap : j * cap + c] = xt_full[:, s : s + c]
        wl = weight[dcore * EPC : (dcore + 1) * EPC]  # [EPC, out, in]
        wt = np.ascontiguousarray(wl.transpose(2, 0, 1).reshape(D, EPC * D))
        in_maps.append({"xt": xt, "wt": wt.astype(np_w)})

    key = (cap, MM_DT, Y_DT, CHUNK, FUSE, WARM, Y_GPSIMD, COPY_SPLIT, XBUFS, YBUFS)
    if key not in _prog_cache:
        _prog_cache[key] = _build_program(cap)
    nc = _prog_cache[key]

    trace = bool(int(os.environ.get("BASSMOE_TRACE", "0")))
    res = run_bass_kernel_spmd(nc, in_maps, list(range(NCORES)), trace=trace)
    last_exec_time_ns = res.exec_time_ns
    last_results = res

    # gather back to token order
    out_t = np.empty((D, T), dtype=np.float32)
    for dcore in range(NCORES):
        yt = np.asarray(res.results[dcore]["yt"]).astype(np.float32)
        for j in range(EPC):
            e = dcore * EPC + j
            s, c = int(starts[e]), int(counts[e])
            out_t[:, s : s + c] = yt[:, j * cap : j * cap + c]
    return np.ascontiguousarray(out_t.T)



# revision 7
# speedup vs baseline: 1.4734x; 1.4734x over previous
"""MoE grouped-GEMM (FMoELinear) on 8 trn2 NeuronCores.

Strategy (expert parallelism):
  - 32 experts, 8 cores -> 4 experts per core.
  - Tokens arrive pre-sorted by expert; host pads each expert's segment to a
    fixed per-expert capacity CAP (multiple of 128) and ships each core a
    transposed activation panel xt[256, 4*CAP] plus its 4 expert weights
    wt[256, 4*256] laid out as [in_feat, expert*256 + out_feat].
  - Device computes yt[o, t] = sum_i W[e][o, i] * x[t, i] per expert with the
    weight stationary in the PE array:
        lhsT = wt[i_chunk, e*256 + oc*128 : +128]   (128 x 128, stationary)
        rhs  = xt tile    [i_chunk, token span]     (128 x 512, moving)
    accumulating the two i-chunks into PSUM, then casting PSUM->SBUF->HBM.
  - Host gathers the non-padded columns back into token order.

Precision/bandwidth plan: rel-err budget is 2e-2; fp16 x/y achieves ~3e-4,
so stream x (and optionally y) as fp8 E3M4 (~1.3% rms quant noise each) to
halve HBM traffic and SBUF pressure. x is pre-scaled by XSCALE (host folds
it back out of y) so fewer values land in the E3M4 subnormal range. The
PSUM->SBUF casts alternate between the DVE and ACT engines (the cast path,
not DMA, was the old bottleneck); x loads ride the SP ring, y stores the
Pool (gpsimd) ring.
"""

import os
import sys
import types

import ml_dtypes
import numpy as np

import concourse.bacc as bacc
import concourse.mybir as mybir
import concourse.tile as tile
from concourse.bass_utils import run_bass_kernel_spmd


def _ensure_axon_hooks_importable():
    """bass_utils imports antenv.axon_hooks when tracing is requested; some
    images lack that module. Provide a no-op fallback so a stray BASS_TRACE
    env var can't crash the kernel (tracing then degrades gracefully)."""
    try:
        import antenv  # noqa: F401
    except ImportError:
        return
    try:
        import antenv.axon_hooks  # noqa: F401
    except ImportError:
        mod = types.ModuleType("antenv.axon_hooks")
        holder = [None]
        mod.set_axon_ntff_profile_hook = lambda h: holder.__setitem__(0, h)
        mod.get_axon_ntff_profile_hook = lambda: holder[0]
        sys.modules["antenv.axon_hooks"] = mod
        import antenv as _antenv

        _antenv.axon_hooks = mod


_ensure_axon_hooks_importable()

NCORES = 8
D = 256  # in/out feature dim
EPC = 4  # experts per core
CAPGRAN = 128  # capacity granularity (pad each expert to a multiple of this)

# observability for test harness
last_exec_time_ns = None
last_results = None

_prog_cache = {}


def _dt1(name):
    if name == "f32":
        return mybir.dt.float32, np.dtype(np.float32)
    if name == "f32r":
        return mybir.dt.float32r, np.dtype(np.float32)
    if name == "f16":
        return mybir.dt.float16, np.dtype(np.float16)
    if name == "bf16":
        return mybir.dt.bfloat16, np.dtype(ml_dtypes.bfloat16)
    if name == "f8e3":
        return mybir.dt.float8e3, np.dtype(ml_dtypes.float8_e3m4)
    if name == "f8e4":
        return mybir.dt.float8e4, np.dtype(ml_dtypes.float8_e4m3)
    if name == "f8e5":
        return mybir.dt.float8e5, np.dtype(ml_dtypes.float8_e5m2)
    raise ValueError(name)


class _Cfg:
    def __init__(self):
        # "xdt" or "xdt+wdt": moving (x) and stationary (w) matmul dtypes
        self.mm_dt = os.environ.get("BASSMOE_MM_DT", "f8e3+f16")
        self.y_dt = os.environ.get("BASSMOE_Y_DT", "f8e3")
        self.xscale = float(os.environ.get("BASSMOE_XSCALE", "2"))
        self.chunk = int(os.environ.get("BASSMOE_CHUNK", "2048"))
        # cast-engine pattern, cycled per PSUM-bank cast: d=DVE, a=ACT
        self.cast_pat = os.environ.get("BASSMOE_CAST_PAT", "da")
        # engine issuing y stores: gpsimd|scalar|sync|vector
        self.st_eng = os.environ.get("BASSMOE_ST_ENG", "gpsimd")
        self.xbufs = int(os.environ.get("BASSMOE_XBUFS", "6"))
        self.ybufs = int(os.environ.get("BASSMOE_YBUFS", "6"))
        self.psbufs = int(os.environ.get("BASSMOE_PSBUFS", "8"))
        parts = self.mm_dt.split("+")
        self.dt_x, self.np_x = _dt1(parts[0])
        self.dt_w, self.np_w = _dt1(parts[-1])
        self.dt_y, self.np_y = _dt1(self.y_dt)

    def key(self, cap):
        return (
            cap,
            self.mm_dt,
            self.y_dt,
            self.chunk,
            self.cast_pat,
            self.st_eng,
            self.xbufs,
            self.ybufs,
            self.psbufs,
        )


def _chunk_offsets(cap: int, chunk: int):
    """(offset, width) chunks covering [0, cap), width <= chunk."""
    out = []
    off = 0
    while off < cap:
        w = min(chunk, cap - off)
        out.append((off, w))
        off += w
    return out


def _splits(width: int):
    """(offset, width) matmul spans <= 512 covering [0, width)."""
    out = []
    off = 0
    while off < width:
        w = min(512, width - off)
        out.append((off, w))
        off += w
    return out


def _build_program(cfg: _Cfg, cap: int):
    """Build the SPMD Bass program for per-expert capacity `cap` tokens."""
    width = EPC * cap
    CHUNK = cfg.chunk

    nc = bacc.Bacc(
        "TRN2",
        target_bir_lowering=False,
        debug=False,
        enable_asserts=False,
        num_devices=NCORES,
    )
    xt = nc.dram_tensor("xt", [D, width], cfg.dt_x, kind="ExternalInput").ap()
    wt = nc.dram_tensor("wt", [D, EPC * D], cfg.dt_w, kind="ExternalInput").ap()
    yt = nc.dram_tensor("yt", [D, width], cfg.dt_y, kind="ExternalOutput").ap()

    cast_engs = [
        {"d": nc.vector, "a": nc.scalar}[c] for c in cfg.cast_pat
    ]
    st_eng = getattr(nc, cfg.st_eng)

    with tile.TileContext(nc) as tc:
        with (
            tc.tile_pool(name="w", bufs=1) as wpool,
            tc.tile_pool(name="x", bufs=cfg.xbufs) as xpool,
            tc.tile_pool(name="y", bufs=cfg.ybufs) as ypool,
            tc.tile_pool(name="ps", bufs=cfg.psbufs, space="PSUM") as pspool,
        ):
            # stationary weights for the whole kernel: two i-chunks
            # (loaded via gpsimd so they don't head-of-line block the x loads)
            w0 = wpool.tile([128, EPC * D], cfg.dt_w, tag="w0")
            w1 = wpool.tile([128, EPC * D], cfg.dt_w, tag="w1")
            nc.gpsimd.dma_start(out=w0[:], in_=wt[0:128, :])
            nc.gpsimd.dma_start(out=w1[:], in_=wt[128:256, :])

            # DRAM view with both 128-row halves on the same 128 partitions
            xt3 = xt.rearrange("(c p) w -> p c w", c=2)

            castidx = 0
            for e in range(EPC):
                for coff, cw in _chunk_offsets(cap, CHUNK):
                    t0 = e * cap + coff
                    x01 = xpool.tile([128, 2 * CHUNK], cfg.dt_x, tag="x01")
                    nc.sync.dma_start(
                        out=x01[:].rearrange("p (c w) -> p c w", c=2)[:, :, :cw],
                        in_=xt3[:, :, t0 : t0 + cw],
                    )
                    x0 = x01[:, 0:CHUNK]
                    x1 = x01[:, CHUNK : 2 * CHUNK]
                    ysb01 = ypool.tile([128, 2 * CHUNK], cfg.dt_y, tag="y01")
                    for oc in range(2):
                        col = e * D + oc * 128
                        ysb = ysb01[:, oc * CHUNK : (oc + 1) * CHUNK]
                        for soff, sw in _splits(cw):
                            ps = pspool.tile([128, 512], mybir.dt.float32, tag="ps")
                            nc.tensor.matmul(
                                ps[:, :sw],
                                w0[:, col : col + 128],
                                x0[:, soff : soff + sw],
                                start=True,
                                stop=False,
                            )
                            nc.tensor.matmul(
                                ps[:, :sw],
                                w1[:, col : col + 128],
                                x1[:, soff : soff + sw],
                                start=False,
                                stop=True,
                            )
                            eng = cast_engs[castidx % len(cast_engs)]
                            castidx += 1
                            if eng is nc.scalar:
                                eng.copy(ysb[:, soff : soff + sw], ps[:, :sw])
                            else:
                                eng.tensor_copy(ysb[:, soff : soff + sw], ps[:, :sw])
                        # store this oc-half as soon as its casts are done
                        st_eng.dma_start(
                            out=yt[oc * 128 : (oc + 1) * 128, t0 : t0 + cw],
                            in_=ysb[:, :cw],
                        )
    nc.compile()
    return nc


def kernel(inp, weight, fwd_expert_count, capacity):
    global last_exec_time_ns, last_results

    cfg = _Cfg()
    inp = np.asarray(inp)
    weight = np.asarray(weight)
    counts = np.asarray(fwd_expert_count).astype(np.int64)
    T, d_in = inp.shape
    E = weight.shape[0]
    assert d_in == D and E == NCORES * EPC
    assert int(counts.sum()) == T, "counts must cover all tokens"

    ends = np.cumsum(counts)
    starts = ends - counts
    cap = max(CAPGRAN, int(-(-int(counts.max()) // CAPGRAN)) * CAPGRAN)
    width = EPC * cap

    # host-side scatter: transpose once, then contiguous row-slice copies
    xt_full = np.ascontiguousarray(inp.T)  # [D, T] float32
    if cfg.xscale != 1.0:
        xt_full = xt_full * np.float32(cfg.xscale)
    if cfg.np_x != np.float32:
        if cfg.np_x.itemsize == 1:
            xt_full = np.clip(xt_full, -15.5, 15.5)
        xt_full = xt_full.astype(cfg.np_x)

    in_maps = []
    for dcore in range(NCORES):
        xt = np.zeros((D, width), dtype=cfg.np_x)
        for j in range(EPC):
            e = dcore * EPC + j
            s, c = int(starts[e]), int(counts[e])
            xt[:, j * cap : j * cap + c] = xt_full[:, s : s + c]
        wl = weight[dcore * EPC : (dcore + 1) * EPC]  # [EPC, out, in]
        wt = np.ascontiguousarray(wl.transpose(2, 0, 1).reshape(D, EPC * D))
        if cfg.xscale != 1.0:
            # x ships as xscale*x; fold 1/xscale into w so PSUM holds
            # unscaled y (e3m4 y-cast must stay within +-15.5)
            wt = wt * np.float32(1.0 / cfg.xscale)
        in_maps.append({"xt": xt, "wt": wt.astype(cfg.np_w)})

    key = cfg.key(cap)
    if key not in _prog_cache:
        _prog_cache[key] = _build_program(cfg, cap)
    nc = _prog_cache[key]

    trace = bool(int(os.environ.get("BASSMOE_TRACE", "0")))
    res = run_bass_kernel_spmd(nc, in_maps, list(range(NCORES)), trace=trace)
    last_exec_time_ns = res.exec_time_ns
    last_results = res

    # gather back to token order (y is unscaled: 1/xscale is folded into w)
    out_t = np.empty((D, T), dtype=np.float32)
    for dcore in range(NCORES):
        yt = np.asarray(res.results[dcore]["yt"]).astype(np.float32)
        for j in range(EPC):
            e = dcore * EPC + j
            s, c = int(starts[e]), int(counts[e])
            out_t[:, s : s + c] = yt[:, j * cap : j * cap + c]
    return np.ascontiguousarray(out_t.T)


# revision 8
# speedup vs baseline: 1.5050x; 1.0214x over previous
"""MoE grouped-GEMM (FMoELinear) on 8 trn2 NeuronCores.

Strategy (expert parallelism):
  - 32 experts, 8 cores -> 4 experts per core.
  - Tokens arrive pre-sorted by expert; host pads each expert's segment to a
    fixed per-expert capacity CAP (multiple of 128) and ships each core a
    transposed activation panel plus its 4 expert weights wt[256, 4*256]
    laid out as [in_feat, expert*256 + out_feat].
  - Device computes yt[o, t] = sum_i W[e][o, i] * x[t, i] per expert with the
    weight stationary in the PE array:
        lhsT = wt[i_chunk, e*256 + oc*128 : +128]   (128 x 128, stationary)
        rhs  = xt tile    [i_chunk, token span]     (128 x 512, moving)
    accumulating the two i-chunks into PSUM, then casting PSUM->SBUF->HBM.
  - Host gathers the non-padded columns back into token order.

Precision/bandwidth plan: rel-err budget is 2e-2; fp16 x/y achieves ~3e-4,
so stream x and y as fp8 E3M4 (~1.3% rms quant noise each; measured total
1.75e-2) to halve HBM traffic and SBUF pressure. x is pre-scaled by XSCALE
(folded back via w/XSCALE) so fewer values land in the E3M4 subnormal range
while PSUM y stays unscaled (|y|max ~8.9 must fit E3M4's +-15.5 on the cast).

Engine layout: PE does 2 matmuls per 512-token span (K=256 split in two
128-row chunks) at 1 cycle/column; the PSUM->SBUF casts alternate between
DVE and ACT (the cast path binds before DMA does); x loads ride the SP
HWDGE ring, y stores the Pool (gpsimd) SWDGE ring, and the two weight
panels load in parallel on the ACT and Pool rings at startup. A short burst
of dummy matmuls during the DMA-warmup window brings the PE out of its low
power-state before real data arrives.

DRAM layout: x and y are stored chunk-major as [128, nblk*2*CHUNK] so every
full-chunk DMA is a single 2*CHUNK-byte contiguous run per partition (bigger
SDMA descriptors -> better per-queue DMA throughput). Block c of expert e
holds tokens [coff, coff+cw) as [2, cw]: row 0 = in-features 0..127, row 1 =
in-features 128..255 (for y: out-features).
"""

import os
import sys
import types

import ml_dtypes
import numpy as np

import concourse.bacc as bacc
import concourse.mybir as mybir
import concourse.tile as tile
from concourse.bass_utils import run_bass_kernel_spmd


def _ensure_axon_hooks_importable():
    """bass_utils imports antenv.axon_hooks when tracing is requested; some
    images lack that module. Provide a no-op fallback so a stray BASS_TRACE
    env var can't crash the kernel (tracing then degrades gracefully)."""
    try:
        import antenv  # noqa: F401
    except ImportError:
        return
    try:
        import antenv.axon_hooks  # noqa: F401
    except ImportError:
        mod = types.ModuleType("antenv.axon_hooks")
        holder = [None]
        mod.set_axon_ntff_profile_hook = lambda h: holder.__setitem__(0, h)
        mod.get_axon_ntff_profile_hook = lambda: holder[0]
        sys.modules["antenv.axon_hooks"] = mod
        import antenv as _antenv

        _antenv.axon_hooks = mod


_ensure_axon_hooks_importable()

NCORES = 8
D = 256  # in/out feature dim
EPC = 4  # experts per core
CAPGRAN = 128  # capacity granularity (pad each expert to a multiple of this)

# observability for test harness
last_exec_time_ns = None
last_results = None

_prog_cache = {}


def _dt1(name):
    if name == "f32":
        return mybir.dt.float32, np.dtype(np.float32)
    if name == "f32r":
        return mybir.dt.float32r, np.dtype(np.float32)
    if name == "f16":
        return mybir.dt.float16, np.dtype(np.float16)
    if name == "bf16":
        return mybir.dt.bfloat16, np.dtype(ml_dtypes.bfloat16)
    if name == "f8e3":
        return mybir.dt.float8e3, np.dtype(ml_dtypes.float8_e3m4)
    if name == "f8e4":
        return mybir.dt.float8e4, np.dtype(ml_dtypes.float8_e4m3)
    if name == "f8e5":
        return mybir.dt.float8e5, np.dtype(ml_dtypes.float8_e5m2)
    raise ValueError(name)


class _Cfg:
    def __init__(self):
        # "xdt" or "xdt+wdt": moving (x) and stationary (w) matmul dtypes
        self.mm_dt = os.environ.get("BASSMOE_MM_DT", "f8e3+f16")
        self.y_dt = os.environ.get("BASSMOE_Y_DT", "f8e3")
        self.xscale = float(os.environ.get("BASSMOE_XSCALE", "2"))
        self.chunk = int(os.environ.get("BASSMOE_CHUNK", "2048"))
        # cast-engine pattern, cycled per PSUM-bank cast: d=DVE, a=ACT
        self.cast_pat = os.environ.get("BASSMOE_CAST_PAT", "da")
        # engine issuing y stores: gpsimd|scalar|sync
        self.st_eng = os.environ.get("BASSMOE_ST_ENG", "gpsimd")
        self.xbufs = int(os.environ.get("BASSMOE_XBUFS", "6"))
        self.ybufs = int(os.environ.get("BASSMOE_YBUFS", "6"))
        self.psbufs = int(os.environ.get("BASSMOE_PSBUFS", "8"))
        self.warm_mms = int(os.environ.get("BASSMOE_WARM_MMS", "5"))
        parts = self.mm_dt.split("+")
        self.dt_x, self.np_x = _dt1(parts[0])
        self.dt_w, self.np_w = _dt1(parts[-1])
        self.dt_y, self.np_y = _dt1(self.y_dt)

    def key(self, cap):
        return (
            cap,
            self.mm_dt,
            self.y_dt,
            self.chunk,
            self.cast_pat,
            self.st_eng,
            self.xbufs,
            self.ybufs,
            self.psbufs,
            self.warm_mms,
        )


def _chunk_offsets(cap: int, chunk: int):
    """(offset, width) chunks covering [0, cap), width <= chunk."""
    out = []
    off = 0
    while off < cap:
        w = min(chunk, cap - off)
        out.append((off, w))
        off += w
    return out


def _splits(width: int):
    """(offset, width) matmul spans <= 512 covering [0, width)."""
    out = []
    off = 0
    while off < width:
        w = min(512, width - off)
        out.append((off, w))
        off += w
    return out


def _build_program(cfg: _Cfg, cap: int):
    """Build the SPMD Bass program for per-expert capacity `cap` tokens."""
    width = EPC * cap
    CHUNK = cfg.chunk

    nc = bacc.Bacc(
        "TRN2",
        target_bir_lowering=False,
        debug=False,
        enable_asserts=False,
        num_devices=NCORES,
    )
    # chunk-major layout: [128, 2*width]; see module docstring
    xt = nc.dram_tensor("xt", [128, 2 * width], cfg.dt_x, kind="ExternalInput").ap()
    wt = nc.dram_tensor("wt", [D, EPC * D], cfg.dt_w, kind="ExternalInput").ap()
    yt = nc.dram_tensor("yt", [128, 2 * width], cfg.dt_y, kind="ExternalOutput").ap()

    cast_engs = [{"d": nc.vector, "a": nc.scalar}[c] for c in cfg.cast_pat]
    st_eng = getattr(nc, cfg.st_eng)

    with tile.TileContext(nc) as tc:
        with (
            tc.tile_pool(name="w", bufs=1) as wpool,
            tc.tile_pool(name="x", bufs=cfg.xbufs) as xpool,
            tc.tile_pool(name="y", bufs=cfg.ybufs) as ypool,
            tc.tile_pool(name="ps", bufs=cfg.psbufs, space="PSUM") as pspool,
        ):
            # stationary weights for the whole kernel: two i-chunks, loaded in
            # parallel on the ACT and Pool rings (SP is busy prefetching x)
            w0 = wpool.tile([128, EPC * D], cfg.dt_w, tag="w0")
            w1 = wpool.tile([128, EPC * D], cfg.dt_w, tag="w1")
            nc.scalar.dma_start(out=w0[:], in_=wt[0:128, :])
            nc.gpsimd.dma_start(out=w1[:], in_=wt[128:256, :])

            # dummy matmuls during the DMA-warmup window pull the PE out of
            # its low p-state before the first real chunk lands
            if cfg.warm_mms:
                wdum = wpool.tile([128, 16], cfg.dt_w, tag="wdum")
                xdum = wpool.tile([128, 512], cfg.dt_x, tag="xdum")
                nc.gpsimd.memset(wdum[:], 0)
                nc.gpsimd.memset(xdum[:], 0)
                for _ in range(cfg.warm_mms):
                    ps = pspool.tile([128, 512], mybir.dt.float32, tag="ps")
                    nc.tensor.matmul(
                        ps[0:16, :], wdum[:], xdum[:], start=True, stop=True
                    )

            castidx = 0
            for e in range(EPC):
                for coff, cw in _chunk_offsets(cap, CHUNK):
                    b0 = 2 * (e * cap + coff)  # flat block offset
                    x01 = xpool.tile([128, 2 * CHUNK], cfg.dt_x, tag="x01")
                    if cw == CHUNK:
                        nc.sync.dma_start(
                            out=x01[:], in_=xt[:, b0 : b0 + 2 * CHUNK]
                        )
                    else:
                        nc.sync.dma_start(
                            out=x01[:].rearrange("p (c w) -> p c w", c=2)[
                                :, :, :cw
                            ],
                            in_=xt[:, b0 : b0 + 2 * cw].rearrange(
                                "p (c w) -> p c w", c=2
                            ),
                        )
                    x0 = x01[:, 0:CHUNK]
                    x1 = x01[:, CHUNK : 2 * CHUNK]
                    ysb01 = ypool.tile([128, 2 * CHUNK], cfg.dt_y, tag="y01")
                    for oc in range(2):
                        col = e * D + oc * 128
                        ysb = ysb01[:, oc * CHUNK : (oc + 1) * CHUNK]
                        for soff, sw in _splits(cw):
                            ps = pspool.tile([128, 512], mybir.dt.float32, tag="ps")
                            nc.tensor.matmul(
                                ps[:, :sw],
                                w0[:, col : col + 128],
                                x0[:, soff : soff + sw],
                                start=True,
                                stop=False,
                            )
                            nc.tensor.matmul(
                                ps[:, :sw],
                                w1[:, col : col + 128],
                                x1[:, soff : soff + sw],
                                start=False,
                                stop=True,
                            )
                            eng = cast_engs[castidx % len(cast_engs)]
                            castidx += 1
                            if eng is nc.scalar:
                                eng.copy(ysb[:, soff : soff + sw], ps[:, :sw])
                            else:
                                eng.tensor_copy(ysb[:, soff : soff + sw], ps[:, :sw])
                    # single store per chunk (both oc halves)
                    if cw == CHUNK:
                        st_eng.dma_start(
                            out=yt[:, b0 : b0 + 2 * CHUNK], in_=ysb01[:]
                        )
                    else:
                        st_eng.dma_start(
                            out=yt[:, b0 : b0 + 2 * cw].rearrange(
                                "p (c w) -> p c w", c=2
                            ),
                            in_=ysb01[:].rearrange("p (c w) -> p c w", c=2)[
                                :, :, :cw
                            ],
                        )
    nc.compile()
    return nc


def kernel(inp, weight, fwd_expert_count, capacity):
    global last_exec_time_ns, last_results

    cfg = _Cfg()
    inp = np.asarray(inp)
    weight = np.asarray(weight)
    counts = np.asarray(fwd_expert_count).astype(np.int64)
    T, d_in = inp.shape
    E = weight.shape[0]
    assert d_in == D and E == NCORES * EPC
    assert int(counts.sum()) == T, "counts must cover all tokens"

    ends = np.cumsum(counts)
    starts = ends - counts
    cap = max(CAPGRAN, int(-(-int(counts.max()) // CAPGRAN)) * CAPGRAN)
    width = EPC * cap
    chunks = _chunk_offsets(cap, cfg.chunk)

    # host-side scatter: transpose once, then contiguous row-slice copies
    xt_full = np.ascontiguousarray(inp.T)  # [D, T] float32
    if cfg.xscale != 1.0:
        xt_full = xt_full * np.float32(cfg.xscale)
    if cfg.np_x != np.float32:
        if cfg.np_x.itemsize == 1:
            xt_full = np.clip(xt_full, -15.5, 15.5)
        xt_full = xt_full.astype(cfg.np_x)

    in_maps = []
    for dcore in range(NCORES):
        # per-expert padded panel [D, width] in the old orientation
        xo = np.zeros((D, width), dtype=cfg.np_x)
        for j in range(EPC):
            e = dcore * EPC + j
            s, c = int(starts[e]), int(counts[e])
            xo[:, j * cap : j * cap + c] = xt_full[:, s : s + c]
        # chunk-major device layout [128, 2*width]
        xd = np.empty((128, 2 * width), dtype=cfg.np_x)
        for j in range(EPC):
            for coff, cw in chunks:
                b0 = 2 * (j * cap + coff)
                t0 = j * cap + coff
                blk = xd[:, b0 : b0 + 2 * cw].reshape(128, 2, cw)
                blk[:, 0, :] = xo[0:128, t0 : t0 + cw]
                blk[:, 1, :] = xo[128:256, t0 : t0 + cw]
        wl = weight[dcore * EPC : (dcore + 1) * EPC]  # [EPC, out, in]
        wt = np.ascontiguousarray(wl.transpose(2, 0, 1).reshape(D, EPC * D))
        if cfg.xscale != 1.0:
            # x ships as xscale*x; fold 1/xscale into w so PSUM holds
            # unscaled y (e3m4 y-cast must stay within +-15.5)
            wt = wt * np.float32(1.0 / cfg.xscale)
        in_maps.append({"xt": xd, "wt": wt.astype(cfg.np_w)})

    key = cfg.key(cap)
    if key not in _prog_cache:
        _prog_cache[key] = _build_program(cfg, cap)
    nc = _prog_cache[key]

    trace = bool(int(os.environ.get("BASSMOE_TRACE", "0")))
    res = run_bass_kernel_spmd(nc, in_maps, list(range(NCORES)), trace=trace)
    last_exec_time_ns = res.exec_time_ns
    last_results = res

    # gather back to token order (y is unscaled: 1/xscale is folded into w)
    out_t = np.empty((D, T), dtype=np.float32)
    for dcore in range(NCORES):
        yd = np.asarray(res.results[dcore]["yt"]).astype(np.float32)
        for j in range(EPC):
            e = dcore * EPC + j
            s, c = int(starts[e]), int(counts[e])
            done = 0
            for coff, cw in chunks:
                if done >= c:
                    break
                take = min(cw, c - done)
                b0 = 2 * (j * cap + coff)
                blk = yd[:, b0 : b0 + 2 * cw].reshape(128, 2, cw)
                out_t[0:128, s + done : s + done + take] = blk[:, 0, :take]
                out_t[128:256, s + done : s + done + take] = blk[:, 1, :take]
                done += take
    return np.ascontiguousarray(out_t.T)


# revision 9
# speedup vs baseline: 1.5401x; 1.0233x over previous
"""MoE grouped-GEMM (FMoELinear) on 8 trn2 NeuronCores.

Strategy (expert parallelism):
  - 32 experts, 8 cores -> 4 experts per core.
  - Tokens arrive pre-sorted by expert; host pads each expert's segment to a
    fixed per-expert capacity CAP (multiple of 128) and ships each core a
    transposed activation panel plus its 4 expert weights wt[256, 4*256]
    laid out as [in_feat, expert*256 + out_feat].
  - Device computes yt[o, t] = sum_i W[e][o, i] * x[t, i] per expert with the
    weight stationary in the PE array:
        lhsT = wt[i_chunk, e*256 + oc*128 : +128]   (128 x 128, stationary)
        rhs  = xt tile    [i_chunk, token span]     (128 x 512, moving)
    accumulating the two i-chunks into PSUM, then casting PSUM->SBUF->HBM.
  - Host gathers the non-padded columns back into token order.

Precision/bandwidth plan: rel-err budget is 2e-2; fp16 x/y achieves ~3e-4,
so stream x and y as fp8 E3M4 (~1.3% rms quant noise each; measured total
1.75e-2) to halve HBM traffic and SBUF pressure. x is pre-scaled by XSCALE
(folded back via w/XSCALE) so fewer values land in the E3M4 subnormal range
while PSUM y stays unscaled (|y|max ~8.9 must fit E3M4's +-15.5 on the cast).

Engine layout: PE does 2 matmuls per 512-token span (K=256 split in two
128-row chunks) at 1 cycle/column; the PSUM->SBUF casts alternate between
DVE and ACT (the cast path binds before DMA does); x loads ride the SP
HWDGE ring, y stores the Pool (gpsimd) SWDGE ring, and the two weight
panels load in parallel on the ACT and Pool rings at startup. A short burst
of dummy matmuls during the DMA-warmup window brings the PE out of its low
power-state before real data arrives.

DRAM layout: x and y are stored chunk-major as [128, nblk*2*CHUNK] so every
full-chunk DMA is a single 2*CHUNK-byte contiguous run per partition (bigger
SDMA descriptors -> better per-queue DMA throughput). Block c of expert e
holds tokens [coff, coff+cw) as [2, cw]: row 0 = in-features 0..127, row 1 =
in-features 128..255 (for y: out-features).
"""

import os
import sys
import types

import ml_dtypes
import numpy as np

import concourse.bacc as bacc
import concourse.mybir as mybir
import concourse.tile as tile
from concourse.bass_utils import run_bass_kernel_spmd


def _ensure_axon_hooks_importable():
    """bass_utils imports antenv.axon_hooks when tracing is requested; some
    images lack that module. Provide a no-op fallback so a stray BASS_TRACE
    env var can't crash the kernel (tracing then degrades gracefully)."""
    try:
        import antenv  # noqa: F401
    except ImportError:
        return
    try:
        import antenv.axon_hooks  # noqa: F401
    except ImportError:
        mod = types.ModuleType("antenv.axon_hooks")
        holder = [None]
        mod.set_axon_ntff_profile_hook = lambda h: holder.__setitem__(0, h)
        mod.get_axon_ntff_profile_hook = lambda: holder[0]
        sys.modules["antenv.axon_hooks"] = mod
        import antenv as _antenv

        _antenv.axon_hooks = mod


_ensure_axon_hooks_importable()

NCORES = 8
D = 256  # in/out feature dim
EPC = 4  # experts per core
CAPGRAN = 128  # capacity granularity (pad each expert to a multiple of this)

# observability for test harness
last_exec_time_ns = None
last_results = None

_prog_cache = {}


def _dt1(name):
    if name == "f32":
        return mybir.dt.float32, np.dtype(np.float32)
    if name == "f32r":
        return mybir.dt.float32r, np.dtype(np.float32)
    if name == "f16":
        return mybir.dt.float16, np.dtype(np.float16)
    if name == "bf16":
        return mybir.dt.bfloat16, np.dtype(ml_dtypes.bfloat16)
    if name == "f8e3":
        return mybir.dt.float8e3, np.dtype(ml_dtypes.float8_e3m4)
    if name == "f8e4":
        return mybir.dt.float8e4, np.dtype(ml_dtypes.float8_e4m3)
    if name == "f8e5":
        return mybir.dt.float8e5, np.dtype(ml_dtypes.float8_e5m2)
    raise ValueError(name)


class _Cfg:
    def __init__(self):
        # "xdt" or "xdt+wdt": moving (x) and stationary (w) matmul dtypes
        self.mm_dt = os.environ.get("BASSMOE_MM_DT", "f8e3+f16")
        self.y_dt = os.environ.get("BASSMOE_Y_DT", "f8e3")
        self.xscale = float(os.environ.get("BASSMOE_XSCALE", "2"))
        self.chunk = int(os.environ.get("BASSMOE_CHUNK", "2048"))
        # cast-engine pattern, cycled per PSUM-bank cast: d=DVE, a=ACT
        self.cast_pat = os.environ.get("BASSMOE_CAST_PAT", "da")
        # engines issuing y stores, cycled per chunk: g=gpsimd, a=ACT, s=SP
        self.st_pat = os.environ.get("BASSMOE_ST_PAT", "ga")
        self.xbufs = int(os.environ.get("BASSMOE_XBUFS", "10"))
        self.ybufs = int(os.environ.get("BASSMOE_YBUFS", "6"))
        self.psbufs = int(os.environ.get("BASSMOE_PSBUFS", "8"))
        self.warm_mms = int(os.environ.get("BASSMOE_WARM_MMS", "7"))
        parts = self.mm_dt.split("+")
        self.dt_x, self.np_x = _dt1(parts[0])
        self.dt_w, self.np_w = _dt1(parts[-1])
        self.dt_y, self.np_y = _dt1(self.y_dt)

    def key(self, cap):
        return (
            cap,
            self.mm_dt,
            self.y_dt,
            self.chunk,
            self.cast_pat,
            self.st_pat,
            self.xbufs,
            self.ybufs,
            self.psbufs,
            self.warm_mms,
            self.st_pat,
        )


def _chunk_offsets(cap: int, chunk: int):
    """(offset, width) chunks covering [0, cap), width <= chunk."""
    out = []
    off = 0
    while off < cap:
        w = min(chunk, cap - off)
        out.append((off, w))
        off += w
    return out


def _splits(width: int):
    """(offset, width) matmul spans <= 512 covering [0, width)."""
    out = []
    off = 0
    while off < width:
        w = min(512, width - off)
        out.append((off, w))
        off += w
    return out


def _build_program(cfg: _Cfg, cap: int):
    """Build the SPMD Bass program for per-expert capacity `cap` tokens."""
    width = EPC * cap
    CHUNK = cfg.chunk

    nc = bacc.Bacc(
        "TRN2",
        target_bir_lowering=False,
        debug=False,
        enable_asserts=False,
        num_devices=NCORES,
    )
    # chunk-major layout: [128, 2*width]; see module docstring
    xt = nc.dram_tensor("xt", [128, 2 * width], cfg.dt_x, kind="ExternalInput").ap()
    wt = nc.dram_tensor("wt", [D, EPC * D], cfg.dt_w, kind="ExternalInput").ap()
    yt = nc.dram_tensor("yt", [128, 2 * width], cfg.dt_y, kind="ExternalOutput").ap()

    cast_engs = [{"d": nc.vector, "a": nc.scalar}[c] for c in cfg.cast_pat]
    st_engs = [
        {"g": nc.gpsimd, "a": nc.scalar, "s": nc.sync}[c] for c in cfg.st_pat
    ]

    with tile.TileContext(nc) as tc:
        with (
            tc.tile_pool(name="w", bufs=1) as wpool,
            tc.tile_pool(name="x", bufs=cfg.xbufs) as xpool,
            tc.tile_pool(name="y", bufs=cfg.ybufs) as ypool,
            tc.tile_pool(name="ps", bufs=cfg.psbufs, space="PSUM") as pspool,
        ):
            # stationary weights for the whole kernel: two i-chunks, loaded in
            # parallel on the ACT and Pool rings (SP is busy prefetching x)
            w0 = wpool.tile([128, EPC * D], cfg.dt_w, tag="w0")
            w1 = wpool.tile([128, EPC * D], cfg.dt_w, tag="w1")
            nc.scalar.dma_start(out=w0[:], in_=wt[0:128, :])
            nc.gpsimd.dma_start(out=w1[:], in_=wt[128:256, :])

            # dummy matmuls during the DMA-warmup window pull the PE out of
            # its low p-state before the first real chunk lands
            if cfg.warm_mms:
                wdum = wpool.tile([128, 16], cfg.dt_w, tag="wdum")
                xdum = wpool.tile([128, 512], cfg.dt_x, tag="xdum")
                nc.gpsimd.memset(wdum[:], 0)
                nc.gpsimd.memset(xdum[:], 0)
                for _ in range(cfg.warm_mms):
                    ps = pspool.tile([128, 512], mybir.dt.float32, tag="ps")
                    nc.tensor.matmul(
                        ps[0:16, :], wdum[:], xdum[:], start=True, stop=True
                    )

            castidx = 0
            chidx = 0
            for e in range(EPC):
                for coff, cw in _chunk_offsets(cap, CHUNK):
                    b0 = 2 * (e * cap + coff)  # flat block offset
                    x01 = xpool.tile([128, 2 * CHUNK], cfg.dt_x, tag="x01")
                    if cw == CHUNK:
                        nc.sync.dma_start(
                            out=x01[:], in_=xt[:, b0 : b0 + 2 * CHUNK]
                        )
                    else:
                        nc.sync.dma_start(
                            out=x01[:].rearrange("p (c w) -> p c w", c=2)[
                                :, :, :cw
                            ],
                            in_=xt[:, b0 : b0 + 2 * cw].rearrange(
                                "p (c w) -> p c w", c=2
                            ),
                        )
                    x0 = x01[:, 0:CHUNK]
                    x1 = x01[:, CHUNK : 2 * CHUNK]
                    ysb01 = ypool.tile([128, 2 * CHUNK], cfg.dt_y, tag="y01")
                    for oc in range(2):
                        col = e * D + oc * 128
                        ysb = ysb01[:, oc * CHUNK : (oc + 1) * CHUNK]
                        for soff, sw in _splits(cw):
                            ps = pspool.tile([128, 512], mybir.dt.float32, tag="ps")
                            nc.tensor.matmul(
                                ps[:, :sw],
                                w0[:, col : col + 128],
                                x0[:, soff : soff + sw],
                                start=True,
                                stop=False,
                            )
                            nc.tensor.matmul(
                                ps[:, :sw],
                                w1[:, col : col + 128],
                                x1[:, soff : soff + sw],
                                start=False,
                                stop=True,
                            )
                            eng = cast_engs[castidx % len(cast_engs)]
                            castidx += 1
                            if eng is nc.scalar:
                                eng.copy(ysb[:, soff : soff + sw], ps[:, :sw])
                            else:
                                eng.tensor_copy(ysb[:, soff : soff + sw], ps[:, :sw])
                    # single store per chunk (both oc halves); alternate
                    # rings so no single DMA queue limits the drain
                    st_eng = st_engs[chidx % len(st_engs)]
                    chidx += 1
                    if cw == CHUNK:
                        st_eng.dma_start(
                            out=yt[:, b0 : b0 + 2 * CHUNK], in_=ysb01[:]
                        )
                    else:
                        st_eng.dma_start(
                            out=yt[:, b0 : b0 + 2 * cw].rearrange(
                                "p (c w) -> p c w", c=2
                            ),
                            in_=ysb01[:].rearrange("p (c w) -> p c w", c=2)[
                                :, :, :cw
                            ],
                        )
    nc.compile()
    return nc


def kernel(inp, weight, fwd_expert_count, capacity):
    global last_exec_time_ns, last_results

    cfg = _Cfg()
    inp = np.asarray(inp)
    weight = np.asarray(weight)
    counts = np.asarray(fwd_expert_count).astype(np.int64)
    T, d_in = inp.shape
    E = weight.shape[0]
    assert d_in == D and E == NCORES * EPC
    assert int(counts.sum()) == T, "counts must cover all tokens"

    ends = np.cumsum(counts)
    starts = ends - counts
    cap = max(CAPGRAN, int(-(-int(counts.max()) // CAPGRAN)) * CAPGRAN)
    width = EPC * cap
    chunks = _chunk_offsets(cap, cfg.chunk)

    # host-side scatter: transpose once, then contiguous row-slice copies
    xt_full = np.ascontiguousarray(inp.T)  # [D, T] float32
    if cfg.xscale != 1.0:
        xt_full = xt_full * np.float32(cfg.xscale)
    if cfg.np_x != np.float32:
        if cfg.np_x.itemsize == 1:
            xt_full = np.clip(xt_full, -15.5, 15.5)
        xt_full = xt_full.astype(cfg.np_x)

    in_maps = []
    for dcore in range(NCORES):
        # per-expert padded panel [D, width] in the old orientation
        xo = np.zeros((D, width), dtype=cfg.np_x)
        for j in range(EPC):
            e = dcore * EPC + j
            s, c = int(starts[e]), int(counts[e])
            xo[:, j * cap : j * cap + c] = xt_full[:, s : s + c]
        # chunk-major device layout [128, 2*width]
        xd = np.empty((128, 2 * width), dtype=cfg.np_x)
        for j in range(EPC):
            for coff, cw in chunks:
                b0 = 2 * (j * cap + coff)
                t0 = j * cap + coff
                blk = xd[:, b0 : b0 + 2 * cw].reshape(128, 2, cw)
                blk[:, 0, :] = xo[0:128, t0 : t0 + cw]
                blk[:, 1, :] = xo[128:256, t0 : t0 + cw]
        wl = weight[dcore * EPC : (dcore + 1) * EPC]  # [EPC, out, in]
        wt = np.ascontiguousarray(wl.transpose(2, 0, 1).reshape(D, EPC * D))
        if cfg.xscale != 1.0:
            # x ships as xscale*x; fold 1/xscale into w so PSUM holds
            # unscaled y (e3m4 y-cast must stay within +-15.5)
            wt = wt * np.float32(1.0 / cfg.xscale)
        in_maps.append({"xt": xd, "wt": wt.astype(cfg.np_w)})

    key = cfg.key(cap)
    if key not in _prog_cache:
        _prog_cache[key] = _build_program(cfg, cap)
    nc = _prog_cache[key]

    trace = bool(int(os.environ.get("BASSMOE_TRACE", "0")))
    res = run_bass_kernel_spmd(nc, in_maps, list(range(NCORES)), trace=trace)
    last_exec_time_ns = res.exec_time_ns
    last_results = res

    # gather back to token order (y is unscaled: 1/xscale is folded into w)
    out_t = np.empty((D, T), dtype=np.float32)
    for dcore in range(NCORES):
        yd = np.asarray(res.results[dcore]["yt"]).astype(np.float32)
        for j in range(EPC):
            e = dcore * EPC + j
            s, c = int(starts[e]), int(counts[e])
            done = 0
            for coff, cw in chunks:
                if done >= c:
                    break
                take = min(cw, c - done)
                b0 = 2 * (j * cap + coff)
                blk = yd[:, b0 : b0 + 2 * cw].reshape(128, 2, cw)
                out_t[0:128, s + done : s + done + take] = blk[:, 0, :take]
                out_t[128:256, s + done : s + done + take] = blk[:, 1, :take]
                done += take
    return np.ascontiguousarray(out_t.T)
